# revision 32
# baseline (speedup 1.0000x reference)
"""Multi-head attention forward on 8 Trainium2 NeuronCores.

Problem: B=4, S=2048, E=1024, H=16, D=64 (fp32 in/out).

Sharding: 8 cores = (batch b, sequence half). Each core owns 1024 query
rows (host passes only those rows as xb). K/V projections are computed for
the LOCAL half only and exchanged with the paired core via a 2-core DRAM
AllGather — softmax over keys is permutation invariant, and both cores
consume the gathered [even-half, odd-half] key order identically, so the
SPMD program needs no per-core specialization.

All matmuls run in bf16 (inputs host-cast; fp32 PSUM accumulation). Layouts
avoid all on-chip transposes: x^T arrives via DMA-transpose, K^T/Q^T in [n, s]
form, V in [s, n] form with a ones column per head (softmax denominator from
the same matmul as attn@V), scores built transposed ([k, q]).

Fully software-pipelined single pass:
  - weights are DMA'd once (per-output-block chunks), not per s-chunk
  - projections run nb-major so head pair j's K/Q/V complete early and
    attention windows (scores -> exp -> attn@V) interleave with remaining
    projections; the Scalar-engine exp stream (the ~293us floor) hides
    under PE work
  - scores PSUM is head-pair packed [128, 2, 512] so one ACT covers both
    heads and the two 64-row score matmuls can overlap via PE row tiling
  - attention windows run qc-major so the first half of the O-projection
    overlaps the second attention sweep
"""

import os
import sys
import types

import numpy as np

sys.path.insert(0, "/opt/trn_rl_repo")

B, S, E, H = 4, 2048, 1024, 16
D = E // H          # 64
Q = S // 2          # query rows per core
NCORES = 8

_compiled = None


def _install_prof_hook():
    try:
        import antenv.axon_hooks  # noqa: F401
        return
    except ImportError:
        pass
    try:
        import antenv
        from trn_agent_boot.trn_boot import _ntff_profile_via_ctypes
    except ImportError:
        return
    mod = types.ModuleType("antenv.axon_hooks")
    mod._hook = None
    mod.set_axon_ntff_profile_hook = lambda h: setattr(mod, "_hook", h)
    mod.get_axon_ntff_profile_hook = lambda: mod._hook
    sys.modules["antenv.axon_hooks"] = mod
    antenv.axon_hooks = mod
    try:
        mod._hook = _ntff_profile_via_ctypes("/opt/axon/libaxon_pjrt.so")
    except Exception:
        mod._hook = None


def _build():
    from contextlib import ExitStack

    from concourse import bacc
    import concourse.mybir as mybir
    from concourse import tile_utils
    from concourse.tile import TileContext

    tile_utils.max_sbuf_usage = 207 * 1024

    F32 = mybir.dt.float32
    BF16 = mybir.dt.bfloat16
    Exp = mybir.ActivationFunctionType.Exp
    Bypass = mybir.AluOpType.bypass

    nc = bacc.Bacc("TRN2", target_bir_lowering=False, debug=False)

    xb = nc.dram_tensor("xb", [Q, E], BF16, kind="ExternalInput")
    wq = nc.dram_tensor("wq", [E, E], BF16, kind="ExternalInput")
    wk = nc.dram_tensor("wk", [E, E], BF16, kind="ExternalInput")
    wv = nc.dram_tensor("wv", [E, E], BF16, kind="ExternalInput")
    wo = nc.dram_tensor("wo", [E, E], BF16, kind="ExternalInput")
    y = nc.dram_tensor("y", [Q, E], F32, kind="ExternalOutput")

    wq_v = wq.ap().rearrange("(eb p) n -> p eb n", p=128)   # [128, 8, 1024]
    wk_v = wk.ap().rearrange("(eb p) n -> p eb n", p=128)
    wv_v = wv.ap().rearrange("(eb p) n -> p eb n", p=128)
    wo_v = wo.ap().rearrange("(eb p) n -> p eb n", p=128)
    y_v = y.ap().rearrange("(sb p) e -> sb p e", p=128)     # [8, 128, 1024]

    EB = E // 128        # 8 e-chunks (contraction blocks)
    NB = E // 128        # 8 n-blocks = head pairs
    KB = S // 128        # 16 key blocks
    QB = Q // 128        # 8 query blocks
    LSB = Q // 128       # 8 local s-blocks (this core's half)
    PAIRS = [[2 * i, 2 * i + 1] for i in range(4)]
    inv_sqrt_d = 1.0 / float(np.sqrt(D))

    with TileContext(nc) as tc:
        with ExitStack() as es:
            xTp = es.enter_context(tc.tile_pool(name="xT", bufs=1))
            kTp = es.enter_context(tc.tile_pool(name="kT", bufs=1))
            qTp = es.enter_context(tc.tile_pool(name="qT", bufs=1))
            vp = es.enter_context(tc.tile_pool(name="vA", bufs=1))
            ctxp = es.enter_context(tc.tile_pool(name="ctx", bufs=1))
            attnp = es.enter_context(tc.tile_pool(name="attn", bufs=4))
            wkqp = es.enter_context(tc.tile_pool(name="wkq", bufs=4))
            wvop = es.enter_context(tc.tile_pool(name="wvo", bufs=2))
            stp = es.enter_context(tc.tile_pool(name="st", bufs=4))
            drp = es.enter_context(tc.tile_pool(name="dr", bufs=3, space="DRAM"))
            gop = es.enter_context(tc.tile_pool(name="go", bufs=1, space="DRAM"))
            ytp = es.enter_context(tc.tile_pool(name="yt", bufs=2))
            nrmp = es.enter_context(tc.tile_pool(name="nrm", bufs=2))
            stgp = es.enter_context(tc.tile_pool(name="stg", bufs=2))
            psP = es.enter_context(tc.tile_pool(name="psP", bufs=2, space="PSUM"))
            psS = es.enter_context(tc.tile_pool(name="psS", bufs=2, space="PSUM"))
            psC = es.enter_context(tc.tile_pool(name="psC", bufs=2, space="PSUM"))

            xT = xTp.tile([128, EB, Q], BF16)        # x^T  [e, s-local]
            kT = kTp.tile([128, NB, S], BF16)        # K^T  [n, s]
            qT = qTp.tile([128, NB, Q], BF16)        # Q^T  [n, q]
            # V with a ones column per head (softmax denominator row)
            vA = vp.tile([128, KB, H, D + 1], BF16)
            ctx = ctxp.tile([128, EB, Q], BF16)      # ctx^T [e, q]

            wkc = [None] * NB
            wqc = [None] * NB

            def load_wk(nb):
                t = wkqp.tile([128, EB, 128], BF16, tag="wkq", name=f"wk{nb}")
                nc.scalar.dma_start(t[:], wk_v[:, :, nb * 128:(nb + 1) * 128])
                wkc[nb] = t

            def load_wq(nb):
                t = wkqp.tile([128, EB, 128], BF16, tag="wkq", name=f"wq{nb}")
                nc.scalar.dma_start(t[:], wq_v[:, :, nb * 128:(nb + 1) * 128])
                wqc[nb] = t

            def kq_proj(nb):
                # local K^T half (s 0:1024) -> pair AllGather -> full kT row
                gin = drp.tile([2, 128, 512], BF16, tag="gik", name=f"gik{nb}")
                gout = gop.tile([2, 2, 128, 512], BF16, tag="go", name=f"gok{nb}")
                for sc in range(2):
                    ps = psP.tile([128, 512], F32, tag="pp", name=f"pk{nb}_{sc}")
                    for eb in range(EB):
                        nc.tensor.matmul(ps[:], wkc[nb][:, eb, :],
                                         xT[:, eb, sc * 512:(sc + 1) * 512],
                                         start=(eb == 0), stop=(eb == EB - 1))
                    st = stp.tile([128, 512], BF16, tag="st", name=f"ks{nb}_{sc}")
                    nc.vector.tensor_copy(st[:], ps[:])
                    nc.sync.dma_start(gin[sc], st[:])
                nc.gpsimd.collective_compute(
                    "AllGather", Bypass, replica_groups=PAIRS,
                    ins=[gin.opt()], outs=[gout.opt()])
                # [half, sc, p, s] -> kT[:, nb, (half sc s)]
                nc.sync.dma_start(
                    kT[:, nb, :].rearrange("p (a b s) -> p a b s", a=2, b=2),
                    gout.rearrange("a b p s -> p a b s"))
                for sc in range(2):
                    ps = psP.tile([128, 512], F32, tag="pp", name=f"pq{nb}_{sc}")
                    for eb in range(EB):
                        nc.tensor.matmul(ps[:], wqc[nb][:, eb, :],
                                         xT[:, eb, sc * 512:(sc + 1) * 512],
                                         start=(eb == 0), stop=(eb == EB - 1))
                    nc.vector.tensor_copy(qT[:, nb, sc * 512:(sc + 1) * 512], ps[:])

            def v_proj(nc2):
                wvc = wvop.tile([128, EB, 512], BF16, tag="wvo", name=f"wv{nc2}")
                nc.sync.dma_start(
                    wvc[:], wv_v[:, :, nc2 * 512:(nc2 + 1) * 512])
                gin = drp.tile([LSB, 128, 512], BF16, tag="giv", name=f"giv{nc2}")
                gout = gop.tile([2, LSB, 128, 512], BF16, tag="go",
                                name=f"gov{nc2}")
                for sb in range(LSB):
                    ps = psP.tile([128, 512], F32, tag="pp", name=f"pv{nc2}_{sb}")
                    for eb in range(EB):
                        nc.tensor.matmul(ps[:],
                                         xT[:, eb, sb * 128:(sb + 1) * 128],
                                         wvc[:, eb, :],
                                         start=(eb == 0), stop=(eb == EB - 1))
                    st = stp.tile([128, 512], BF16, tag="st", name=f"vs{nc2}_{sb}")
                    nc.vector.tensor_copy(st[:], ps[:])
                    nc.sync.dma_start(gin[sb], st[:])
                nc.gpsimd.collective_compute(
                    "AllGather", Bypass, replica_groups=PAIRS,
                    ins=[gin.opt()], outs=[gout.opt()])
                for half in range(2):
                    for lsb in range(LSB):
                        # [p, (h d)] -> vA[:, half*8 + lsb, nc2*8 + h, d]
                        nc.sync.dma_start(
                            vA[:, half * LSB + lsb,
                               nc2 * 8:(nc2 + 1) * 8, 0:D],
                            gout[half, lsb].rearrange("p (h d) -> p h d", d=D))

            def attn_window(j, qc):
                # scores^T + exp + attn@V for head pair j, query chunk qc
                qs = slice(qc * 512, (qc + 1) * 512)
                quarters = [
                    attnp.tile([128, 4, 2, 512], BF16, tag="attn",
                               name=f"at{j}_{qc}_{i}") for i in range(4)]
                for kb in range(KB):
                    sps = psS.tile([128, 2, 512], F32, tag="sps",
                                   name=f"sc{j}_{qc}_{kb}")
                    for hh in range(2):
                        p0 = hh * 64
                        nc.tensor.matmul(
                            sps[:, hh, :],
                            kT[p0:p0 + 64, j, kb * 128:(kb + 1) * 128],
                            qT[p0:p0 + 64, j, qs],
                            start=True, stop=True)
                    nc.scalar.activation(
                        quarters[kb // 4][:, kb % 4, :, :]
                        .rearrange("p a b -> p (a b)"),
                        sps.rearrange("p a b -> p (a b)"), Exp,
                        scale=inv_sqrt_d)

                cpss = [psC.tile([128, 512], F32, tag="cps",
                                 name=f"cp{j}_{qc}_{i}") for i in range(2)]
                for kb in range(KB):
                    for hh in range(2):
                        nc.tensor.matmul(
                            cpss[hh][0:D + 1, :],
                            vA[:, kb, 2 * j + hh, :],
                            quarters[kb // 4][:, kb % 4, hh, :],
                            start=(kb == 0), stop=(kb == KB - 1))
                for hh in range(2):
                    # stage PSUM -> SBUF so the cps banks recycle early
                    cpb = nrmp.tile([D + 1, 512], F32, tag="cpb")
                    nc.vector.tensor_copy(cpb[:], cpss[hh][0:D + 1, :])
                    den = nrmp.tile([1, 512], F32, tag="den")
                    nc.vector.tensor_copy(den[:], cpb[D:D + 1, :])
                    nc.vector.reciprocal_approx_fast(den[:], den[:])
                    bcast = nrmp.tile([64, 512], F32, tag="bc")
                    nc.gpsimd.partition_broadcast(bcast[:], den[:])
                    if hh == 0:
                        nc.vector.tensor_mul(
                            ctx[0:64, j, qs], cpb[0:D, :], bcast[:])
                    else:
                        stg = stgp.tile([64, 512], BF16, tag="stg")
                        nc.vector.tensor_mul(stg[:], cpb[0:D, :], bcast[:])
                        nc.sync.dma_start(ctx[64:128, j, qs], stg[:])

            woc = [None, None]

            def load_wo(nc2):
                t = wvop.tile([128, EB, 512], BF16, tag="wvo", name=f"wo{nc2}")
                nc.sync.dma_start(t[:], wo_v[:, :, nc2 * 512:(nc2 + 1) * 512])
                woc[nc2] = t

            def o_proj(qb_range):
                for nc2 in range(2):
                    for qb in qb_range:
                        ps = psP.tile([128, 512], F32, tag="pp",
                                      name=f"py{nc2}_{qb}")
                        for eb in range(EB):
                            nc.tensor.matmul(ps[:],
                                             ctx[:, eb, qb * 128:(qb + 1) * 128],
                                             woc[nc2][:, eb, :],
                                             start=(eb == 0), stop=(eb == EB - 1))
                        yt = ytp.tile([128, 512], F32)
                        nc.vector.tensor_copy(yt[:], ps[:])
                        nc.sync.dma_start(
                            y_v[qb][:, nc2 * 512:(nc2 + 1) * 512], yt[:])

            # ---------------- emission (priority) order ----------------
            load_wk(0); load_wq(0)
            # x^T via DMA transpose, [512,128] chunks spread over both
            # hwdge queues so the first K-proj group is ready early.
            # CAUTION: emitting these transposes before the first weight
            # DMAs, or using [1024,128] chunks, makes consumers observe
            # stale xT (DMA-transpose sem accounting quirk) - keep this
            # exact size and position.
            for sc in range(2):
                for eb in range(EB):
                    eng = nc.sync if eb % 2 == 0 else nc.scalar
                    eng.dma_start_transpose(
                        xT[:, eb, sc * 512:(sc + 1) * 512],
                        xb.ap()[sc * 512:(sc + 1) * 512,
                                eb * 128:(eb + 1) * 128])
            nc.gpsimd.memset(vA[:, :, :, D], 1.0)    # ones column (all heads)
            load_wk(1); load_wq(1)
            kq_proj(0)
            v_proj(0)
            kq_proj(1)
            # qc-major attention sweeps; proj for pair nb emitted just ahead
            for j in range(NB):
                if j + 2 < NB:
                    load_wk(j + 2); load_wq(j + 2)
                    kq_proj(j + 2)
                if j == 2:
                    v_proj(1)
                attn_window(j, 0)
            load_wo(0)
            load_wo(1)
            for j in range(NB):
                attn_window(j, 1)
                if j == 6:
                    o_proj(range(0, 4))      # q 0:512 ready after qc=0 sweep
            o_proj(range(4, QB))

    nc.compile()
    return nc


def kernel(x, Wq, Wk, Wv, Wo):
    global _compiled
    _install_prof_hook()
    import ml_dtypes
    from concourse import bass_utils

    if _compiled is None:
        _compiled = _build()
    nc = _compiled

    bf16 = ml_dtypes.bfloat16
    x = np.ascontiguousarray(x, dtype=np.float32)
    wq_b = np.ascontiguousarray(np.asarray(Wq, dtype=np.float32).astype(bf16))
    wk_b = np.ascontiguousarray(np.asarray(Wk, dtype=np.float32).astype(bf16))
    wv_b = np.ascontiguousarray(np.asarray(Wv, dtype=np.float32).astype(bf16))
    wo_b = np.ascontiguousarray(np.asarray(Wo, dtype=np.float32).astype(bf16))

    in_maps = []
    for c in range(NCORES):
        b, half = c // 2, c % 2
        xc = x[b, half * Q:(half + 1) * Q]   # this core's query rows
        in_maps.append({
            "xb": np.ascontiguousarray(xc.astype(bf16)),
            "wq": wq_b, "wk": wk_b, "wv": wv_b, "wo": wo_b,
        })

    trace = bool(int(os.environ.get("KERNEL_TRACE", "0")))
    res = bass_utils.run_bass_kernel_spmd(
        nc, in_maps, core_ids=list(range(NCORES)), trace=trace)
    kernel.last_result = res

    out = np.empty((B, S, E), dtype=np.float32)
    for c in range(NCORES):
        b, half = c // 2, c % 2
        out[b, half * Q:(half + 1) * Q] = res.results[c]["y"]
    return out


kernel.last_result = None


# revision 34
# speedup vs baseline: 1.0379x; 1.0379x over previous
"""Multi-head attention forward on 8 Trainium2 NeuronCores.

Problem: B=4, S=2048, E=1024, H=16, D=64 (fp32 in/out).

Sharding: 8 cores = (batch b, sequence half). Each core owns 1024 query
rows (host passes only those rows as xb). K/V projections are computed for
the LOCAL half only and exchanged with the paired core via a 2-core DRAM
AllGather — softmax over keys is permutation invariant, and both cores
consume the gathered [even-half, odd-half] key order identically, so the
SPMD program needs no per-core specialization.

All matmuls run in bf16 (inputs host-cast; fp32 PSUM accumulation). Layouts
avoid all on-chip transposes: x^T arrives via DMA-transpose, K^T/Q^T in [n, s]
form, V in [s, n] form with a ones column per head (softmax denominator from
the same matmul as attn@V), scores built transposed ([k, q]).

Fully software-pipelined single pass:
  - weights are DMA'd once (per-output-block chunks), not per s-chunk
  - projections run nb-major so head pair j's K/Q/V complete early and
    attention windows (scores -> exp -> attn@V) interleave with remaining
    projections; the Scalar-engine exp stream (the ~293us floor) hides
    under PE work
  - scores PSUM is head-pair packed [128, 2, 512] so one ACT covers both
    heads and the two 64-row score matmuls can overlap via PE row tiling
  - attention windows run qc-major so the first half of the O-projection
    overlaps the second attention sweep
"""

import os
import sys
import types

import numpy as np

sys.path.insert(0, "/opt/trn_rl_repo")

B, S, E, H = 4, 2048, 1024, 16
D = E // H          # 64
Q = S // 2          # query rows per core
NCORES = 8

_compiled = None


def _install_prof_hook():
    try:
        import antenv.axon_hooks  # noqa: F401
        return
    except ImportError:
        pass
    try:
        import antenv
        from trn_agent_boot.trn_boot import _ntff_profile_via_ctypes
    except ImportError:
        return
    mod = types.ModuleType("antenv.axon_hooks")
    mod._hook = None
    mod.set_axon_ntff_profile_hook = lambda h: setattr(mod, "_hook", h)
    mod.get_axon_ntff_profile_hook = lambda: mod._hook
    sys.modules["antenv.axon_hooks"] = mod
    antenv.axon_hooks = mod
    try:
        mod._hook = _ntff_profile_via_ctypes("/opt/axon/libaxon_pjrt.so")
    except Exception:
        mod._hook = None


def _build():
    from contextlib import ExitStack

    from concourse import bacc
    import concourse.mybir as mybir
    from concourse import tile_utils
    from concourse.tile import TileContext

    tile_utils.max_sbuf_usage = 207 * 1024

    F32 = mybir.dt.float32
    BF16 = mybir.dt.bfloat16
    Exp = mybir.ActivationFunctionType.Exp
    Bypass = mybir.AluOpType.bypass

    nc = bacc.Bacc("TRN2", target_bir_lowering=False, debug=False)

    xb = nc.dram_tensor("xb", [Q, E], BF16, kind="ExternalInput")
    wq = nc.dram_tensor("wq", [E, E], BF16, kind="ExternalInput")
    wk = nc.dram_tensor("wk", [E, E], BF16, kind="ExternalInput")
    wv = nc.dram_tensor("wv", [E, E], BF16, kind="ExternalInput")
    wo = nc.dram_tensor("wo", [E, E], BF16, kind="ExternalInput")
    y = nc.dram_tensor("y", [Q, E], F32, kind="ExternalOutput")

    wq_v = wq.ap().rearrange("(eb p) n -> p eb n", p=128)   # [128, 8, 1024]
    wk_v = wk.ap().rearrange("(eb p) n -> p eb n", p=128)
    wv_v = wv.ap().rearrange("(eb p) n -> p eb n", p=128)
    wo_v = wo.ap().rearrange("(eb p) n -> p eb n", p=128)
    y_v = y.ap().rearrange("(sb p) e -> sb p e", p=128)     # [8, 128, 1024]

    EB = E // 128        # 8 e-chunks (contraction blocks)
    NB = E // 128        # 8 n-blocks = head pairs
    KB = S // 128        # 16 key blocks
    QB = Q // 128        # 8 query blocks
    LSB = Q // 128       # 8 local s-blocks (this core's half)
    PAIRS = [[2 * i, 2 * i + 1] for i in range(4)]
    inv_sqrt_d = 1.0 / float(np.sqrt(D))

    with TileContext(nc) as tc:
        with ExitStack() as es:
            xTp = es.enter_context(tc.tile_pool(name="xT", bufs=1))
            kTp = es.enter_context(tc.tile_pool(name="kT", bufs=1))
            qTp = es.enter_context(tc.tile_pool(name="qT", bufs=1))
            vp = es.enter_context(tc.tile_pool(name="vA", bufs=1))
            ctxp = es.enter_context(tc.tile_pool(name="ctx", bufs=1))
            attnp = es.enter_context(tc.tile_pool(name="attn", bufs=4))
            wkqp = es.enter_context(tc.tile_pool(name="wkq", bufs=4))
            wvop = es.enter_context(tc.tile_pool(name="wvo", bufs=2))
            stp = es.enter_context(tc.tile_pool(name="st", bufs=4))
            drp = es.enter_context(tc.tile_pool(name="dr", bufs=3, space="DRAM"))
            gop = es.enter_context(tc.tile_pool(name="go", bufs=1, space="DRAM"))
            ytp = es.enter_context(tc.tile_pool(name="yt", bufs=2))
            nrmp = es.enter_context(tc.tile_pool(name="nrm", bufs=2))
            stgp = es.enter_context(tc.tile_pool(name="stg", bufs=2))
            psP = es.enter_context(tc.tile_pool(name="psP", bufs=2, space="PSUM"))
            psS = es.enter_context(tc.tile_pool(name="psS", bufs=2, space="PSUM"))
            psC = es.enter_context(tc.tile_pool(name="psC", bufs=2, space="PSUM"))

            xT = xTp.tile([128, EB, Q], BF16)        # x^T  [e, s-local]
            kT = kTp.tile([128, NB, S], BF16)        # K^T  [n, s]
            qT = qTp.tile([128, NB, Q], BF16)        # Q^T  [n, q]
            # V with a ones column per head (softmax denominator row)
            vA = vp.tile([128, KB, H, D + 1], BF16)
            ctx = ctxp.tile([128, EB, Q], BF16)      # ctx^T [e, q]

            wkc = [None] * NB
            wqc = [None] * NB

            def load_wk(nb):
                t = wkqp.tile([128, EB, 128], BF16, tag="wkq", name=f"wk{nb}")
                nc.scalar.dma_start(t[:], wk_v[:, :, nb * 128:(nb + 1) * 128])
                wkc[nb] = t

            def load_wq(nb):
                t = wkqp.tile([128, EB, 128], BF16, tag="wkq", name=f"wq{nb}")
                nc.scalar.dma_start(t[:], wq_v[:, :, nb * 128:(nb + 1) * 128])
                wqc[nb] = t

            def kq_proj(nb):
                # local K^T half (s 0:1024) -> pair AllGather -> full kT row
                gin = drp.tile([2, 128, 512], BF16, tag="gik", name=f"gik{nb}")
                gout = gop.tile([2, 2, 128, 512], BF16, tag="go", name=f"gok{nb}")
                for sc in range(2):
                    ps = psP.tile([128, 512], F32, tag="pp", name=f"pk{nb}_{sc}")
                    for eb in range(EB):
                        nc.tensor.matmul(ps[:], wkc[nb][:, eb, :],
                                         xT[:, eb, sc * 512:(sc + 1) * 512],
                                         start=(eb == 0), stop=(eb == EB - 1))
                    st = stp.tile([128, 512], BF16, tag="st", name=f"ks{nb}_{sc}")
                    nc.vector.tensor_copy(st[:], ps[:])
                    nc.sync.dma_start(gin[sc], st[:])
                nc.gpsimd.collective_compute(
                    "AllGather", Bypass, replica_groups=PAIRS,
                    ins=[gin.opt()], outs=[gout.opt()])
                # [half, sc, p, s] -> kT[:, nb, (half sc s)]
                nc.sync.dma_start(
                    kT[:, nb, :].rearrange("p (a b s) -> p a b s", a=2, b=2),
                    gout.rearrange("a b p s -> p a b s"))
                for sc in range(2):
                    ps = psP.tile([128, 512], F32, tag="pp", name=f"pq{nb}_{sc}")
                    for eb in range(EB):
                        nc.tensor.matmul(ps[:], wqc[nb][:, eb, :],
                                         xT[:, eb, sc * 512:(sc + 1) * 512],
                                         start=(eb == 0), stop=(eb == EB - 1))
                    nc.vector.tensor_copy(qT[:, nb, sc * 512:(sc + 1) * 512], ps[:])

            def v_proj(nc2):
                wvc = wvop.tile([128, EB, 512], BF16, tag="wvo", name=f"wv{nc2}")
                nc.sync.dma_start(
                    wvc[:], wv_v[:, :, nc2 * 512:(nc2 + 1) * 512])
                gin = drp.tile([LSB, 128, 512], BF16, tag="giv", name=f"giv{nc2}")
                gout = gop.tile([2, LSB, 128, 512], BF16, tag="go",
                                name=f"gov{nc2}")
                for sb in range(LSB):
                    ps = psP.tile([128, 512], F32, tag="pp", name=f"pv{nc2}_{sb}")
                    for eb in range(EB):
                        nc.tensor.matmul(ps[:],
                                         xT[:, eb, sb * 128:(sb + 1) * 128],
                                         wvc[:, eb, :],
                                         start=(eb == 0), stop=(eb == EB - 1))
                    st = stp.tile([128, 512], BF16, tag="st", name=f"vs{nc2}_{sb}")
                    nc.vector.tensor_copy(st[:], ps[:])
                    nc.sync.dma_start(gin[sb], st[:])
                nc.gpsimd.collective_compute(
                    "AllGather", Bypass, replica_groups=PAIRS,
                    ins=[gin.opt()], outs=[gout.opt()])
                for half in range(2):
                    for lsb in range(LSB):
                        # [p, (h d)] -> vA[:, half*8 + lsb, nc2*8 + h, d]
                        nc.sync.dma_start(
                            vA[:, half * LSB + lsb,
                               nc2 * 8:(nc2 + 1) * 8, 0:D],
                            gout[half, lsb].rearrange("p (h d) -> p h d", d=D))

            def attn_window(j, qc):
                # scores^T + exp + attn@V for head pair j, query chunk qc
                qs = slice(qc * 512, (qc + 1) * 512)
                quarters = [
                    attnp.tile([128, 4, 2, 512], BF16, tag="attn",
                               name=f"at{j}_{qc}_{i}") for i in range(4)]
                for kb in range(KB):
                    sps = psS.tile([128, 2, 512], F32, tag="sps",
                                   name=f"sc{j}_{qc}_{kb}")
                    for hh in range(2):
                        p0 = hh * 64
                        nc.tensor.matmul(
                            sps[:, hh, :],
                            kT[p0:p0 + 64, j, kb * 128:(kb + 1) * 128],
                            qT[p0:p0 + 64, j, qs],
                            start=True, stop=True)
                    nc.scalar.activation(
                        quarters[kb // 4][:, kb % 4, :, :]
                        .rearrange("p a b -> p (a b)"),
                        sps.rearrange("p a b -> p (a b)"), Exp,
                        scale=inv_sqrt_d)

                cpss = [psC.tile([128, 512], F32, tag="cps",
                                 name=f"cp{j}_{qc}_{i}") for i in range(2)]
                for kb in range(KB):
                    for hh in range(2):
                        nc.tensor.matmul(
                            cpss[hh][0:D + 1, :],
                            vA[:, kb, 2 * j + hh, :],
                            quarters[kb // 4][:, kb % 4, hh, :],
                            start=(kb == 0), stop=(kb == KB - 1))
                for hh in range(2):
                    # stage PSUM -> SBUF so the cps banks recycle early
                    cpb = nrmp.tile([D + 1, 512], F32, tag="cpb")
                    nc.vector.tensor_copy(cpb[:], cpss[hh][0:D + 1, :])
                    den = nrmp.tile([1, 512], F32, tag="den")
                    nc.vector.tensor_copy(den[:], cpb[D:D + 1, :])
                    nc.vector.reciprocal_approx_fast(den[:], den[:])
                    bcast = nrmp.tile([64, 512], F32, tag="bc")
                    nc.gpsimd.partition_broadcast(bcast[:], den[:])
                    if hh == 0:
                        nc.vector.tensor_mul(
                            ctx[0:64, j, qs], cpb[0:D, :], bcast[:])
                    else:
                        stg = stgp.tile([64, 512], BF16, tag="stg")
                        nc.vector.tensor_mul(stg[:], cpb[0:D, :], bcast[:])
                        nc.sync.dma_start(ctx[64:128, j, qs], stg[:])

            woc = [None, None]

            def load_wo(nc2):
                t = wvop.tile([128, EB, 512], BF16, tag="wvo", name=f"wo{nc2}")
                nc.sync.dma_start(t[:], wo_v[:, :, nc2 * 512:(nc2 + 1) * 512])
                woc[nc2] = t

            def o_proj(qb_range):
                for nc2 in range(2):
                    for qb in qb_range:
                        ps = psP.tile([128, 512], F32, tag="pp",
                                      name=f"py{nc2}_{qb}")
                        for eb in range(EB):
                            nc.tensor.matmul(ps[:],
                                             ctx[:, eb, qb * 128:(qb + 1) * 128],
                                             woc[nc2][:, eb, :],
                                             start=(eb == 0), stop=(eb == EB - 1))
                        yt = ytp.tile([128, 512], F32)
                        nc.vector.tensor_copy(yt[:], ps[:])
                        nc.sync.dma_start(
                            y_v[qb][:, nc2 * 512:(nc2 + 1) * 512], yt[:])

            # ---------------- emission (priority) order ----------------
            load_wk(0); load_wq(0)
            # x^T via DMA transpose, [512,128] chunks spread over both
            # hwdge queues so the first K-proj group is ready early.
            # CAUTION: emitting these transposes before the first weight
            # DMAs, or using [1024,128] chunks, makes consumers observe
            # stale xT (DMA-transpose sem accounting quirk) - keep this
            # exact size and position.
            for sc in range(2):
                for eb in range(EB):
                    eng = nc.sync if eb % 2 == 0 else nc.scalar
                    eng.dma_start_transpose(
                        xT[:, eb, sc * 512:(sc + 1) * 512],
                        xb.ap()[sc * 512:(sc + 1) * 512,
                                eb * 128:(eb + 1) * 128])
            nc.gpsimd.memset(vA[:, :, :, D], 1.0)    # ones column (all heads)
            load_wk(1); load_wq(1)
            kq_proj(0)
            v_proj(0)
            kq_proj(1)
            # qc-major attention sweeps; proj for pair nb emitted just ahead
            for j in range(NB):
                if j + 2 < NB:
                    load_wk(j + 2); load_wq(j + 2)
                    kq_proj(j + 2)
                if j == 2:
                    v_proj(1)
                attn_window(j, 0)
            load_wo(0)
            load_wo(1)
            for j in range(NB):
                attn_window(j, 1)
                if j == 6:
                    o_proj(range(0, 4))      # q 0:512 ready after qc=0 sweep
            o_proj(range(4, QB))

    nc.compile()
    return nc


def kernel(x, Wq, Wk, Wv, Wo):
    global _compiled
    _install_prof_hook()
    import ml_dtypes
    from concourse import bass_utils

    if _compiled is None:
        _compiled = _build()
    nc = _compiled

    bf16 = ml_dtypes.bfloat16
    x = np.ascontiguousarray(x, dtype=np.float32)
    wq_b = np.ascontiguousarray(np.asarray(Wq, dtype=np.float32).astype(bf16))
    wk_b = np.ascontiguousarray(np.asarray(Wk, dtype=np.float32).astype(bf16))
    wv_b = np.ascontiguousarray(np.asarray(Wv, dtype=np.float32).astype(bf16))
    wo_b = np.ascontiguousarray(np.asarray(Wo, dtype=np.float32).astype(bf16))

    in_maps = []
    for c in range(NCORES):
        b, half = c // 2, c % 2
        xc = x[b, half * Q:(half + 1) * Q]   # this core's query rows
        in_maps.append({
            "xb": np.ascontiguousarray(xc.astype(bf16)),
            "wq": wq_b, "wk": wk_b, "wv": wv_b, "wo": wo_b,
        })

    trace = bool(int(os.environ.get("KERNEL_TRACE", "0")))
    res = bass_utils.run_bass_kernel_spmd(
        nc, in_maps, core_ids=list(range(NCORES)), trace=trace)
    kernel.last_result = res

    out = np.empty((B, S, E), dtype=np.float32)
    for c in range(NCORES):
        b, half = c // 2, c % 2
        out[b, half * Q:(half + 1) * Q] = res.results[c]["y"]
    return out


kernel.last_result = None
